# revision 26
# baseline (speedup 1.0000x reference)
"""YOLO-v2 loss kernel for Trainium2 (8 NeuronCores, data-parallel over batch).

Layout insight: pyolos [B, 425, 26, 26] is [B, ch*5anc, hw] with plane = c*5+a.
The loss needs:
  - conf channel (planes 0..4) densely: sum of sigmoid(conf)^2 over all
    positions (background term; gconf == 0 exactly wherever no GT matched).
  - cls/txywh channels only at the <=8 matched (cell, anchor) slots per image.
So each core reads 16 conf-plane blocks (216KB) + an indirect gather of
128 slots x 90 values instead of the full 18.4MB chunk. The DGE on this
hardware reads, per offset row, OUT_ROW_LEN contiguous elements from
offs[p,0] (per-element offsets are ignored), so the host pre-transposes
pyolos to per-CELL rows [img, cell, 90ch, 5anc] (anchor innermost, the
5 anchor-derived constants interleaved as channels 5..9). The gather row
index needs only the cell -- not the argmax -- so it launches right after
the cheap floor chain while the IoU/argmax runs in parallel; a one-hot
anchor select on-chip then reduces [90,5] -> [90].

Structure:
  - consts + conf DMAs issued BEFORE the TileContext on separate engine
    queues, overlapping the tile preamble; parent-block semaphore waits
    guard the body.
  - floor() via i32 round-trip + compare correction (exact under any
    rounding mode); IoU via reciprocal; fused scalar_tensor_tensor and
    dual-column tensor_scalar ops.
  - cross-slot last-writer/ignore logic via ONE group-spread matmul over a
    block-diagonal 0/1 matrix carrying [key, cell, 13w, 13h, 169*areag];
    the "ignore" predicate re-evaluates iou_wh>0.5 pairwise from the
    spread w/h (no bit extraction).
  - final weighted sums via PE matmuls with the pos/pos*weight masks as
    weight vectors.

Per-core partial sums (7 f32) are combined on the host (the all-reduce-mean
step of the data-parallel recipe).
"""

import numpy as np

from concourse import bass, mybir
from concourse.bass_utils import run_bass_kernel_spmd
from concourse.tile import TileContext

F32 = mybir.dt.float32
I32 = mybir.dt.int32
AF = mybir.ActivationFunctionType
OP = mybir.AluOpType
AX = mybir.AxisListType

NC = 8                 # cores
B = 128                # batch
BL = B // NC           # images per core (16)
NGT = 8                # GTs per image
S = BL * NGT           # slots per core (128)
GRID = 26
HW = GRID * GRID       # 676
NANC = 5
IMG = 425 * HW         # elements per image (287300)
NCHG = 90              # gathered: conf, txywh, 80 cls, 5 anchor-derived
EPS = 1e-7
ANC = np.array([[0.05, 0.07], [0.12, 0.15], [0.25, 0.30],
                [0.45, 0.50], [0.80, 0.85]], np.float32)

# ---- consts tensor column layout ----
C_GBRB = 0      # [r,b,r,b]
C_GBNL = 4      # [-l,-t,l,t]
C_CLIP4 = 8     # [26l, 26t, -26r, -26b]
C_AH5 = 12
C_AW5 = 17
C_AREAEPS = 22  # anchor area + EPS
C_IOTAM99 = 27  # iota5 - 99
C_ROWOFF = 32   # img*676 (row-index base for the cell gather)
C_UT8 = 33      # [j > i%8]
C_MASK8 = 41    # [j == i%8]
C_IOTA80 = 49
C_LBLM1 = 129   # label - 1
C_ONES = 130
C_G = 131       # group-equality matrix [i//8 == j//8], 128 cols
NCONST = 259

# gathered channel order: [conf, tx, ty, tw, th, cls0..cls79, 5 fakes]
CH_ORDER = [0, 81, 82, 83, 84] + list(range(1, 81))


def _make_consts(gbx_core: np.ndarray, lbl_core: np.ndarray) -> np.ndarray:
    ct = np.zeros((S, NCONST), np.float32)
    l, t, r, b = (gbx_core[:, k] for k in range(4))
    ct[:, C_GBRB:C_GBRB + 4] = np.stack([r, b, r, b], 1)
    ct[:, C_GBNL:C_GBNL + 4] = np.stack([-l, -t, l, t], 1)
    ct[:, C_CLIP4:C_CLIP4 + 4] = np.stack([26 * l, 26 * t, -26 * r, -26 * b], 1)
    ct[:, C_AH5:C_AH5 + 5] = ANC[:, 1][None, :]
    ct[:, C_AW5:C_AW5 + 5] = ANC[:, 0][None, :]
    ct[:, C_AREAEPS:C_AREAEPS + 5] = (ANC[:, 0] * ANC[:, 1] + EPS)[None, :]
    ct[:, C_IOTAM99:C_IOTAM99 + 5] = np.arange(5, dtype=np.float32)[None] - 99.0
    i = np.arange(S)
    ct[:, C_ROWOFF] = ((i // NGT) * HW).astype(np.float32)
    j = np.arange(8)
    ct[:, C_UT8:C_UT8 + 8] = (j[None, :] > (i % 8)[:, None]).astype(np.float32)
    ct[:, C_MASK8:C_MASK8 + 8] = (j[None, :] == (i % 8)[:, None]).astype(np.float32)
    ct[:, C_IOTA80:C_IOTA80 + 80] = np.arange(80, dtype=np.float32)[None, :]
    ct[:, C_LBLM1] = lbl_core - 1.0
    ct[:, C_ONES] = 1.0
    ct[:, C_G:C_G + S] = (i[:, None] // 8 == i[None, :] // 8).astype(np.float32)
    return ct


# per-(anchor,cell) gather-row tail: the 5 anchor-derived constants
_ANCTAIL = np.stack([13 * ANC[:, 0], 13 * ANC[:, 1],
                     np.log(ANC[:, 0]), np.log(ANC[:, 1]),
                     169.0 * (ANC[:, 0] * ANC[:, 1] + EPS)], 1)  # [5, 5]


def _split_multiwaits(nc: bass.Bass, k: int = 1) -> None:
    """This walrus build rejects instructions with >~2 sync waits; hoist
    extra waits onto preceding same-engine NoOps (equivalent for monotone
    sem-ge waits)."""
    for fn in nc.m.functions:
        for bb in fn.blocks:
            out = []
            for inst in bb.instructions:
                si = inst.sync_info
                waits = list(si.on_wait) if si is not None and si.on_wait else []
                if len(waits) > k:
                    for i, w in enumerate(waits[:-k]):
                        out.append(mybir.InstNoOp(
                            name=f"{inst.name}-wsplit{i}",
                            engine=inst.engine,
                            bass_nofuse=True,
                            sync_info=mybir.SyncInfo(on_wait=[w],
                                                     on_update=[]),
                        ))
                    inst.sync_info = mybir.SyncInfo(
                        on_wait=waits[-k:], on_update=list(si.on_update))
                out.append(inst)
            bb.instructions = out


_COMPUTE_OPS = (
    mybir.InstTensorTensor, mybir.InstTensorScalarPtr, mybir.InstTensorReduce,
    mybir.InstActivation, mybir.InstMatmult, mybir.InstLdweights,
    mybir.InstTensorCopy,
)


def _check_wait_order(nc: bass.Bass, wait_names: dict) -> None:
    """Assert each engine's dma-wait precedes its first compute op."""
    pos = {}
    first_compute = {}
    idx = 0
    for fn in nc.m.functions:
        for bb in fn.blocks:
            for inst in bb.instructions:
                idx += 1
                eng = inst.engine
                if inst.name in wait_names.values():
                    pos[inst.name] = idx
                elif isinstance(inst, _COMPUTE_OPS) and eng not in first_compute:
                    first_compute[eng] = (idx, inst.name)
    for eng, nm in wait_names.items():
        if nm not in pos:
            raise AssertionError(f"wait {nm} for {eng} not found")
        if eng in first_compute and pos[nm] > first_compute[eng][0]:
            raise AssertionError(
                f"wait {nm} scheduled after first {eng} compute "
                f"{first_compute[eng][1]}")


def build_bass() -> bass.Bass:
    nc = bass.Bass()
    py = nc.declare_dram_parameter("pyt", [BL * HW, NCHG * NANC], F32,
                                   isOutput=False)
    cf = nc.declare_dram_parameter("confp", [BL * 5, HW], F32, isOutput=False)
    cn = nc.declare_dram_parameter("consts", [S, NCONST], F32, isOutput=False)
    out = nc.declare_dram_parameter("out", [1, 96], F32, isOutput=True)

    # static SBUF + early DMAs (before the tile context so the transfers
    # overlap the tile preamble). The sync engine DRAINs its own queue and
    # bumps the semaphore itself: engine-side sem propagation beats the
    # DMA completion-descriptor path (~2.4us) if the drain is fast.
    ct = nc.alloc_sbuf_tensor("ct_static", [S, NCONST], F32)
    conf = nc.alloc_sbuf_tensor("conf_static", [BL * 5, HW], F32)
    sem_ct = nc.alloc_semaphore("dsem_ct")
    sem_cf = nc.alloc_semaphore("dsem_cf")
    nc.sync.dma_start(out=ct[:, :], in_=cn[:, :]).then_inc(sem_ct, 16)
    nc.scalar.dma_start(out=conf[:, :], in_=cf[:, :]).then_inc(sem_cf, 16)
    # parent-block waits: engines that read the static tensors block here,
    # before the tile body; the DMAs run during the engine preambles.
    wv = nc.vector.wait_ge(sem_ct, 16)
    wg = nc.gpsimd.wait_ge(sem_ct, 16)
    wt = nc.tensor.wait_ge(sem_ct, 16)
    ws = nc.scalar.wait_ge(sem_cf, 16)
    wait_names = {
        mybir.EngineType.DVE: wv.ins.name,
        mybir.EngineType.Pool: wg.ins.name,
        mybir.EngineType.PE: wt.ins.name,
        mybir.EngineType.Activation: ws.ins.name,
    }
    with TileContext(nc) as tc:
        with (
            tc.tile_pool(name="sb", bufs=1) as sb,
            tc.tile_pool(name="ps", bufs=1, space="PSUM") as ps,
        ):
            def tt(shape, tag, dt=F32):
                return sb.tile(shape, dt, name=tag)

            # ---------------- dense conf term (scalar engine) -----------
            # sigmoid(x)^2 = exp(-2*softplus(-x)); Exp/Ln only so every ACT
            # shares one table load (the 40-segment Sigmoid table is also
            # too coarse for this 432K-term sum)
            sigc = tt([BL * 5, HW], "sigc")
            densesq = tt([BL * 5, 1], "densesq")
            nc.scalar.activation(sigc[:], conf[:, :], AF.Exp, scale=-1.0)
            nc.scalar.activation(sigc[:], sigc[:], AF.Ln, bias=1.0)
            nc.scalar.activation(sigc[:], sigc[:], AF.Exp, scale=-2.0,
                                 accum_out=densesq[:])

            # ---------------- matching: gather-critical chain (vector) --
            u4 = tt([S, 4], "u4")        # [2cx, 2cy, w, h]
            nc.vector.tensor_tensor(out=u4[:], in0=ct[:, C_GBRB:C_GBRB + 4],
                                    in1=ct[:, C_GBNL:C_GBNL + 4],
                                    op=OP.subtract)
            inh = tt([S, 5], "inh")
            nc.vector.tensor_scalar(inh[:], ct[:, C_AH5:C_AH5 + 5],
                                    u4[:, 3:4], None, OP.min)
            inter = tt([S, 5], "inter")
            nc.vector.scalar_tensor_tensor(out=inter[:],
                                           in0=ct[:, C_AW5:C_AW5 + 5],
                                           scalar=u4[:, 2:3], in1=inh[:],
                                           op0=OP.min, op1=OP.mult)
            areag = tt([S, 1], "areag")
            nc.vector.tensor_tensor(out=areag[:], in0=u4[:, 2:3],
                                    in1=u4[:, 3:4], op=OP.mult)
            den = tt([S, 5], "den")
            nc.vector.scalar_tensor_tensor(out=den[:],
                                           in0=ct[:, C_AREAEPS:C_AREAEPS + 5],
                                           scalar=areag[:, 0:1], in1=inter[:],
                                           op0=OP.add, op1=OP.subtract)
            deni = tt([S, 5], "deni")
            nc.vector.reciprocal(deni[:], den[:])
            iou2 = tt([S, 5], "iou2")
            nc.vector.tensor_tensor(out=iou2[:], in0=inter[:], in1=deni[:],
                                    op=OP.mult)
            mx = tt([S, 1], "mx")
            nc.vector.tensor_reduce(mx[:], iou2[:], AX.X, OP.max)
            eqm = tt([S, 5], "eqm")
            nc.vector.tensor_scalar(eqm[:], iou2[:], mx[:, 0:1], None,
                                    OP.is_equal)
            tsel = tt([S, 5], "tsel")
            nc.vector.tensor_tensor(out=tsel[:], in0=eqm[:],
                                    in1=ct[:, C_IOTAM99:C_IOTAM99 + 5],
                                    op=OP.mult)
            tmin = tt([S, 1], "tmin")    # idx_max - 99
            nc.vector.tensor_reduce(tmin[:], tsel[:], AX.X, OP.min)

            # ---------------- floor chain (gpsimd, parallel) -------------
            # gp computes its own u4 head so it doesn't wait on vector
            u4g = tt([S, 2], "u4g")
            nc.gpsimd.tensor_tensor(out=u4g[:], in0=ct[:, C_GBRB:C_GBRB + 2],
                                    in1=ct[:, C_GBNL:C_GBNL + 2],
                                    op=OP.subtract)
            c26 = tt([S, 2], "c26")
            nc.gpsimd.tensor_scalar(c26[:], u4g[:, 0:2], 13.0, None, OP.mult)
            ci = tt([S, 2], "ci", I32)
            nc.gpsimd.tensor_copy(ci[:], c26[:])
            cif = tt([S, 2], "cif")
            nc.gpsimd.tensor_copy(cif[:], ci[:])
            dtc = tt([S, 2], "dtc")
            nc.gpsimd.tensor_tensor(out=dtc[:], in0=cif[:], in1=c26[:],
                                    op=OP.subtract)
            gtc = tt([S, 2], "gtc")
            nc.gpsimd.tensor_scalar(gtc[:], dtc[:], 0.0, None, OP.is_gt)
            colrow = tt([S, 2], "colrow")
            nc.gpsimd.tensor_tensor(out=colrow[:], in0=cif[:], in1=gtc[:],
                                    op=OP.subtract)
            txy = tt([S, 2], "txy")
            nc.gpsimd.tensor_tensor(out=txy[:], in0=c26[:], in1=colrow[:],
                                    op=OP.subtract)
            cell = tt([S, 1], "cell")
            nc.gpsimd.tensor_scalar(cell[:], colrow[:, 1:2], 26.0,
                                    colrow[:, 0:1], OP.mult, OP.add)
            celloff = tt([S, 1], "celloff")  # cell + img*676 = gather row
            nc.gpsimd.tensor_scalar(celloff[:], cell[:],
                                    ct[:, C_ROWOFF:C_ROWOFF + 1], None,
                                    OP.add)

            # ---------------- cell gather (gpsimd, argmax-independent) ---
            offs_i = tt([S, 1], "offs_i", I32)
            nc.gpsimd.tensor_copy(offs_i[:], celloff[:])
            pf5 = tt([S, NCHG * NANC], "pf5")
            nc.gpsimd.indirect_dma_start(
                out=pf5[:], out_offset=None, in_=py[:, :],
                in_offset=bass.IndirectOffsetOnAxis(ap=offs_i[:, :], axis=0))
            # one-hot anchor select: decode-critical 10 channels first
            oh5 = tt([S, 5], "oh5")
            nc.vector.tensor_scalar(oh5[:], ct[:, C_IOTAM99:C_IOTAM99 + 5],
                                    tmin[:, 0:1], None, OP.is_equal)
            z50 = tt([S, 50], "z50")
            sel10 = tt([S, 10], "sel10")
            with tc.high_priority():
                nc.vector.tensor_tensor(
                    out=z50[:].rearrange("p (c a) -> p c a", a=5),
                    in0=pf5[:, 0:50].rearrange("p (c a) -> p c a", a=5),
                    in1=oh5[:].rearrange("p (o a) -> p o a",
                                         o=1).to_broadcast([S, 10, 5]),
                    op=OP.mult)
                nc.vector.tensor_reduce(
                    sel10[:], z50[:].rearrange("p (c a) -> p c a", a=5),
                    AX.X, OP.add)
            z400 = tt([S, 400], "z400")
            sel80 = tt([S, 80], "sel80")
            nc.vector.tensor_tensor(
                out=z400[:].rearrange("p (c a) -> p c a", a=5),
                in0=pf5[:, 50:450].rearrange("p (c a) -> p c a", a=5),
                in1=oh5[:].rearrange("p (o a) -> p o a",
                                     o=1).to_broadcast([S, 80, 5]),
                op=OP.mult)
            nc.vector.tensor_reduce(
                sel80[:], z400[:].rearrange("p (c a) -> p c a", a=5),
                AX.X, OP.add)

            # ---------------- aux + cross-slot (gather window) -----------
            oh80 = tt([S, 80], "oh80")
            nc.vector.tensor_scalar(oh80[:], ct[:, C_IOTA80:C_IOTA80 + 80],
                                    ct[:, C_LBLM1:C_LBLM1 + 1], None,
                                    OP.is_equal)
            key = tt([S, 1], "key")
            nc.vector.tensor_scalar(key[:], cell[:], 5.0, tmin[:, 0:1],
                                    OP.mult, OP.add)
            wh13 = tt([S, 2], "wh13")
            nc.vector.tensor_scalar(wh13[:], u4[:, 2:4], 13.0, None, OP.mult)
            wcol = tt([S, 4], "wcol")    # [weight, weight+1, 676*areag, 169*areag]
            nc.vector.tensor_scalar(wcol[:, 0:1], areag[:], -1.0, 2.0,
                                    OP.mult, OP.add)
            nc.vector.tensor_scalar(wcol[:, 1:2], areag[:], -1.0, 3.0,
                                    OP.mult, OP.add)
            nc.vector.tensor_scalar(wcol[:, 2:3], areag[:], float(HW), None,
                                    OP.mult)
            nc.vector.tensor_scalar(wcol[:, 3:4], areag[:], 169.0, None,
                                    OP.mult)
            spr = tt([S, 40], "spr")     # [key8, cell8, w8, h8, areag169_8]
            msk = ct[:, C_MASK8:C_MASK8 + 8]
            nc.vector.tensor_tensor(out=spr[:, 0:8], in0=msk,
                                    in1=key[:, 0:1].to_broadcast([S, 8]),
                                    op=OP.mult)
            nc.vector.tensor_tensor(out=spr[:, 8:16], in0=msk,
                                    in1=cell[:, 0:1].to_broadcast([S, 8]),
                                    op=OP.mult)
            nc.vector.tensor_tensor(out=spr[:, 16:24], in0=msk,
                                    in1=wh13[:, 0:1].to_broadcast([S, 8]),
                                    op=OP.mult)
            nc.vector.tensor_tensor(out=spr[:, 24:32], in0=msk,
                                    in1=wh13[:, 1:2].to_broadcast([S, 8]),
                                    op=OP.mult)
            nc.vector.tensor_tensor(out=spr[:, 32:40], in0=msk,
                                    in1=wcol[:, 3:4].to_broadcast([S, 8]),
                                    op=OP.mult)
            grp_p = ps.tile([S, 40], F32, name="grp_p")
            nc.tensor.matmul(out=grp_p[:], lhsT=ct[:, C_G:C_G + S],
                             rhs=spr[:], start=True, stop=True)
            grp = tt([S, 40], "grp")
            nc.vector.tensor_copy(grp[:], grp_p[:])
            eqk = tt([S, 8], "eqk")
            nc.vector.tensor_scalar(eqk[:], grp[:, 0:8], key[:, 0:1], None,
                                    OP.is_equal)
            ovm = tt([S, 8], "ovm")
            nc.vector.tensor_tensor(out=ovm[:], in0=eqk[:],
                                    in1=ct[:, C_UT8:C_UT8 + 8], op=OP.mult)
            ovw = tt([S, 1], "ovw")
            nc.vector.tensor_reduce(ovw[:], ovm[:], AX.X, OP.max)
            lastw = tt([S, 1], "lastw")
            nc.vector.tensor_scalar(lastw[:], ovw[:], -1.0, 1.0,
                                    OP.mult, OP.add)
            eqc = tt([S, 8], "eqc")
            nc.vector.tensor_scalar(eqc[:], grp[:, 8:16], cell[:, 0:1], None,
                                    OP.is_equal)
            em = tt([S, 8], "em")
            nc.vector.tensor_tensor(out=em[:], in0=eqc[:],
                                    in1=ct[:, C_UT8:C_UT8 + 8], op=OP.mult)

            # scalar-engine helper
            lnwh = tt([S, 2], "lnwh")
            nc.scalar.activation(lnwh[:], u4[:, 2:4], AF.Ln)

            # ---------------- post-gather: scalar activations ------------
            u3 = tt([S, 3], "u3")
            nc.scalar.activation(u3[:], sel10[:, 0:3], AF.Exp, scale=-1.0)
            sig3 = tt([S, 3], "sig3")
            nc.vector.tensor_scalar(sig3[:], u3[:], 1.0, None, OP.add)
            nc.vector.reciprocal(sig3[:], sig3[:])
            ewh = tt([S, 2], "ewh")
            nc.scalar.activation(ewh[:], sel10[:, 3:5], AF.Exp)
            vals84 = tt([S, 84], "vals84")  # [dsq, psq, 1, spsum, q80...]
            vals4b = tt([S, 4], "vals4b")   # [sps2, mse_wh, q2x, q2y]
            sp80 = tt([S, 80], "sp80")
            nc.scalar.activation(sp80[:], sel80[:], AF.Exp)
            nc.scalar.activation(sp80[:], sp80[:], AF.Ln, bias=1.0,
                                 accum_out=vals84[:, 3:4])   # softplus sum
            sp2 = tt([S, 2], "sp2")
            nc.scalar.activation(sp2[:], sel10[:, 1:3], AF.Exp)
            nc.scalar.activation(sp2[:], sp2[:], AF.Ln, bias=1.0,
                                 accum_out=vals4b[:, 0:1])

            # ---------------- ignore test (gpsimd, pairwise iou_wh) ------
            ig1 = tt([S, 8], "ig1")
            nc.gpsimd.tensor_scalar(ig1[:], grp[:, 16:24], sel10[:, 5:6],
                                    None, OP.min)
            ig2 = tt([S, 8], "ig2")
            nc.gpsimd.tensor_scalar(ig2[:], grp[:, 24:32], sel10[:, 6:7],
                                    None, OP.min)
            igi = tt([S, 8], "igi")
            nc.gpsimd.tensor_tensor(out=igi[:], in0=ig1[:], in1=ig2[:],
                                    op=OP.mult)
            ig3 = tt([S, 8], "ig3")
            nc.gpsimd.tensor_scalar(ig3[:], igi[:], 3.0, None, OP.mult)
            ig4 = tt([S, 8], "ig4")
            nc.gpsimd.tensor_tensor(out=ig4[:], in0=ig3[:],
                                    in1=grp[:, 32:40], op=OP.subtract)
            mign8 = tt([S, 8], "mign8")
            nc.gpsimd.tensor_scalar(mign8[:], ig4[:], sel10[:, 9:10], None,
                                    OP.is_gt)
            z8 = tt([S, 8], "z8")
            nc.gpsimd.tensor_tensor(out=z8[:], in0=mign8[:], in1=em[:],
                                    op=OP.mult)
            # loss helpers on gpsimd: weighted dot terms go through mm rhs
            nc.gpsimd.tensor_tensor(out=vals84[:, 4:84], in0=oh80[:],
                                    in1=sel80[:], op=OP.mult)
            nc.gpsimd.tensor_tensor(out=vals4b[:, 2:4], in0=txy[:],
                                    in1=sel10[:, 1:3], op=OP.mult)
            twh = tt([S, 2], "twh")
            nc.gpsimd.tensor_tensor(out=twh[:], in0=lnwh[:],
                                    in1=sel10[:, 7:9], op=OP.subtract)
            dwh = tt([S, 2], "dwh")
            nc.gpsimd.tensor_tensor(out=dwh[:], in0=sel10[:, 3:5], in1=twh[:],
                                    op=OP.subtract)

            # ---------------- post-gather: vector decode/IoU -------------
            h2 = tt([S, 2], "h2")        # pwh26/2
            nc.vector.tensor_tensor(out=h2[:], in0=ewh[:], in1=sel10[:, 5:7],
                                    op=OP.mult)
            pxs4 = tt([S, 4], "pxs4")    # [pxy | -pxy]
            nc.vector.tensor_tensor(out=pxs4[:, 0:2], in0=sig3[:, 1:3],
                                    in1=colrow[:], op=OP.add)
            nc.vector.tensor_scalar(pxs4[:, 2:4], pxs4[:, 0:2], -1.0, None,
                                    OP.mult)
            pm4 = tt([S, 4], "pm4")      # [plt | -prb]
            nc.vector.tensor_tensor(
                out=pm4[:].rearrange("p (x y) -> p x y", y=2),
                in0=pxs4[:].rearrange("p (x y) -> p x y", y=2),
                in1=h2[:, 0:2].rearrange("p (o y) -> p o y",
                                         o=1).to_broadcast([S, 2, 2]),
                op=OP.subtract)
            ab4 = tt([S, 4], "ab4")      # [max(plt,glt) | max(-prb,-grb)]
            nc.vector.tensor_tensor(out=ab4[:], in0=pm4[:],
                                    in1=ct[:, C_CLIP4:C_CLIP4 + 4], op=OP.max)
            s2 = tt([S, 2], "s2")        # -(iwh)
            nc.vector.tensor_tensor(out=s2[:], in0=ab4[:, 0:2],
                                    in1=ab4[:, 2:4], op=OP.add)
            nc.vector.tensor_scalar(s2[:], s2[:], 0.0, None, OP.min)
            inter26 = tt([S, 1], "inter26")
            nc.vector.tensor_tensor(out=inter26[:], in0=s2[:, 0:1],
                                    in1=s2[:, 1:2], op=OP.mult)
            mp = tt([S, 1], "mp")        # mask_pos * lastw
            nc.vector.scalar_tensor_tensor(out=mp[:], in0=inter26[:],
                                           scalar=0.0, in1=lastw[:],
                                           op0=OP.is_gt, op1=OP.mult)
            pa4 = tt([S, 1], "pa4")
            nc.gpsimd.tensor_tensor(out=pa4[:], in0=h2[:, 0:1],
                                    in1=h2[:, 1:2], op=OP.mult)
            d2a = tt([S, 1], "d2a")      # pa26 - inter26
            nc.vector.scalar_tensor_tensor(out=d2a[:], in0=pa4[:],
                                           scalar=4.0, in1=inter26[:],
                                           op0=OP.mult, op1=OP.subtract)
            den2 = tt([S, 1], "den2")
            nc.vector.tensor_scalar(den2[:], d2a[:], wcol[:, 2:3],
                                    float(HW) * EPS, OP.add, OP.add)
            rec2 = tt([S, 1], "rec2")
            nc.vector.reciprocal(rec2[:], den2[:])
            gconf = tt([S, 1], "gconf")
            nc.vector.tensor_tensor(out=gconf[:], in0=inter26[:],
                                    in1=rec2[:], op=OP.mult)
            dconf = tt([S, 1], "dconf")
            nc.vector.tensor_scalar(dconf[:], sig3[:, 0:1], gconf[:, 0:1],
                                    None, OP.subtract)

            # masks tail
            ignov = tt([S, 1], "ignov")
            nc.vector.tensor_reduce(ignov[:], z8[:], AX.X, OP.max)
            wt8 = tt([S, 1], "wt8")
            nc.vector.tensor_tensor(out=wt8[:], in0=ignov[:],
                                    in1=wcol[:, 1:2], op=OP.mult)
            weff = tt([S, 1], "weff")
            nc.vector.tensor_tensor(out=weff[:], in0=wcol[:, 0:1],
                                    in1=wt8[:], op=OP.subtract)
            mpw = tt([S, 1], "mpw")
            nc.vector.tensor_tensor(out=mpw[:], in0=mp[:], in1=weff[:],
                                    op=OP.mult)

            # vals columns
            nc.vector.memset(vals84[:, 2:3], 1.0)
            nc.scalar.activation(vals84[:, 0:1], dconf[:], AF.Square)
            nc.scalar.activation(vals84[:, 1:2], sig3[:, 0:1], AF.Square)
            dw2 = tt([S, 2], "dw2")
            nc.scalar.activation(dw2[:], dwh[:], AF.Square,
                                 accum_out=vals4b[:, 1:2])

            # ---------------- final weighted reductions (PE) -------------
            red = ps.tile([1, 96], F32, name="red")
            nc.tensor.matmul(out=red[0:1, 0:84], lhsT=mp[:], rhs=vals84[:],
                             start=True, stop=True)
            nc.tensor.matmul(out=red[0:1, 84:88], lhsT=mpw[:], rhs=vals4b[:],
                             start=True, stop=True)
            nc.tensor.matmul(out=red[0:1, 88:89],
                             lhsT=ct[0:BL * 5, C_ONES:C_ONES + 1],
                             rhs=densesq[:], start=True, stop=True)
            osb = tt([1, 96], "osb")
            nc.vector.memset(osb[:], 0.0)
            nc.vector.tensor_copy(osb[0:1, 0:89], red[0:1, 0:89])
            nc.sync.dma_start(out=out[:, :], in_=osb[:])
    _check_wait_order(nc, wait_names)
    _split_multiwaits(nc, k=1)
    return nc


_NC_CACHE = None
LAST_RESULTS = None


def _get_nc():
    global _NC_CACHE
    if _NC_CACHE is None:
        _NC_CACHE = build_bass()
    return _NC_CACHE


def make_core_inputs(py, gbx, lbl, c):
    sl = slice(c * BL, (c + 1) * BL)
    pyc = py[sl].reshape(BL, 85, NANC, HW)
    pyt = np.empty((BL, HW, NCHG, NANC), np.float32)
    pyt[:, :, 0:5, :] = pyc[:, [0, 81, 82, 83, 84]].transpose(0, 3, 1, 2)
    pyt[:, :, 5:10, :] = _ANCTAIL.T[None, None, :, :]
    pyt[:, :, 10:90, :] = pyc[:, 1:81].transpose(0, 3, 1, 2)
    return {
        "pyt": pyt.reshape(BL * HW, NCHG * NANC),
        "confp": np.ascontiguousarray(pyc[:, 0].reshape(BL * 5, HW)),
        "consts": _make_consts(gbx[sl].reshape(S, 4), lbl[sl].reshape(S)),
    }


def run(pyolos, gboxes_ltrb, labels, trace=False, **spmd_kwargs):
    global LAST_RESULTS
    nc = _get_nc()
    py = np.asarray(pyolos, np.float32).reshape(B, 425, HW)
    gbx = np.asarray(gboxes_ltrb, np.float32)
    lbl = np.asarray(labels).astype(np.float32)
    in_maps = [make_core_inputs(py, gbx, lbl, c) for c in range(NC)]
    res = run_bass_kernel_spmd(nc, in_maps, list(range(NC)), trace=trace,
                               **spmd_kwargs)
    LAST_RESULTS = res
    outs = np.stack([r["out"][0] for r in res.results]).astype(np.float64)
    t = outs.sum(0)
    pos_mse, pos_psq, npos = t[0], t[1], t[2]
    cls_num = t[3] - t[4:84].sum()
    txy_s = t[84] - (t[86] + t[87])
    twh_s = t[85]
    dense_sq = t[88]
    loss = (5.0 * pos_mse / B
            + (dense_sq - pos_psq) / B
            + cls_num / max(npos, 1.0)
            + txy_s / B
            + twh_s / B)
    return np.float32(loss)


def kernel(pyolos, gboxes_ltrb, labels):
    return run(pyolos, gboxes_ltrb, labels)
